# revision 4
# baseline (speedup 1.0000x reference)
"""Multi-head attention Trainium2 kernel (8 NeuronCores, SPMD).

Problem: N=2, Lq=Lk=2048, D=1024, H=16 heads, causal + padding mask,
score scaling = sqrt(#valid keys per sentence).

Sharding: core c -> (n = c // 4, g = c % 4): batch n, head group g of 4
heads (256 feature columns). No cross-core communication; the host
assembles the per-core [2048, 256] outputs into [2, 2048, 1024].

Per-core pipeline (all on one NeuronCore):
  1. PE-transpose xq/xk (fp32r) into d-major slabs.
  2. Projections (fp32r matmuls): QT/KT [256f, 2048s] (stored bf16) and
     V [2048s, 256f] -> packed bf16 Vtilde [k-chunk][128, 4*65] with a
     ones column per head (gives softmax denominators for free).
  3. Scores transposed ST[k, q] = KT.T-slices @ QT (bf16, row-tiled head
     pairs). Causal handled structurally (skip fully-masked chunks) plus
     one [128,128] triangular additive mask on diagonal chunks. Padding
     mask enters as the scalar-engine activation bias (per-partition = per
     key). exp on ACT writes bf16 P'T tiles.
  4. PV: out[q, 0:64]+sum[q] = P'T-chunk.T @ Vtilde (bf16), accumulated
     over k-chunks in PSUM; normalize by the ones-column; DMA out.
"""

import sys

sys.path.insert(0, "/opt/trn_rl_repo")

import numpy as np

import concourse.bass as bass  # noqa: F401  (bass types used via nc)
import concourse.tile as tile
from concourse import bacc, mybir
from concourse.bass_utils import run_bass_kernel_spmd

F32 = mybir.dt.float32
F32R = mybir.dt.float32r
BF16 = mybir.dt.bfloat16

L = 2048          # sequence length (q and k)
D = 1024          # model dim
FPC = 256         # features per core (4 heads x 64)
HPC = 4           # heads per core
DH = 64           # head dim
SC = L // 128     # 16 seq chunks of 128
DC = D // 128     # 8 d chunks of 128
NB = L // 512     # 4 q-blocks of 512
NEG = -1.0e9


def build_program():
    nc = bacc.Bacc("TRN2", target_bir_lowering=False, debug=False, num_devices=8)

    xq_d = nc.dram_tensor("xq", [L, D], F32R, kind="ExternalInput").ap()
    xk_d = nc.dram_tensor("xk", [L, D], F32R, kind="ExternalInput").ap()
    wq_d = nc.dram_tensor("wq_t", [D, FPC], F32R, kind="ExternalInput").ap()
    wk_d = nc.dram_tensor("wk_t", [D, FPC], F32R, kind="ExternalInput").ap()
    wv_d = nc.dram_tensor("wv_t", [D, FPC], F32R, kind="ExternalInput").ap()
    pb_d = nc.dram_tensor("pad_bias", [128, SC], F32, kind="ExternalInput").ap()
    id_d = nc.dram_tensor("ident", [128, 128], F32R, kind="ExternalInput").ap()
    out_d = nc.dram_tensor("out", [L, FPC], F32, kind="ExternalOutput").ap()

    with tile.TileContext(nc) as tc:
        with (
            tc.tile_pool(name="consts", bufs=1) as consts,
            tc.tile_pool(name="wpool", bufs=1) as wpool,
            tc.tile_pool(name="qkv", bufs=1) as qkv,
        ):
            ident = consts.tile([128, 128], F32R)
            nc.sync.dma_start(out=ident, in_=id_d)
            # diag_mask[i, j] = NEG where j < i else 0  (strictly-lower tri)
            diag_mask = consts.tile([128, 128], F32)
            nc.gpsimd.memset(diag_mask, 0.0)
            nc.gpsimd.affine_select(
                out=diag_mask,
                in_=diag_mask,
                compare_op=mybir.AluOpType.is_ge,
                fill=NEG,
                base=0,
                pattern=[[1, 128]],
                channel_multiplier=-1,
            )
            pad_bias = consts.tile([128, SC], F32)
            nc.sync.dma_start(out=pad_bias, in_=pb_d)

            # weights: [128 (d within chunk), dc, f]
            wq = wpool.tile([128, DC, FPC], F32R)
            wk = wpool.tile([128, DC, FPC], F32R)
            wv = wpool.tile([128, DC, FPC], F32R)
            for w_sb, w_dr in ((wq, wq_d), (wk, wk_d), (wv, wv_d)):
                nc.sync.dma_start(
                    out=w_sb, in_=w_dr.rearrange("(dc p) f -> p dc f", p=128)
                )

            # projection outputs
            qt = qkv.tile([128, 2, L], BF16)   # [f within chunk, fc, q]
            kt = qkv.tile([128, 2, L], BF16)   # [f within chunk, fc, k]
            vt = qkv.tile([128, SC, HPC * 65], BF16)  # [k in chunk, kc, h*65+f]
            nc.vector.memset(vt, 1.0)  # ones columns (col 64 of each head)

            # ---- phase 1+2: transpose + projections, streamed in 512-row slabs
            with (
                tc.tile_pool(name="xin", bufs=2) as xin_pool,
                tc.tile_pool(name="slab", bufs=2) as slab_pool,
                tc.tile_pool(name="tpsum", bufs=2, space="PSUM") as tpsum_pool,
                tc.tile_pool(name="ppsum", bufs=2, space="PSUM") as ppsum_pool,
                tc.tile_pool(name="vpsum", bufs=2, space="PSUM") as vpsum_pool,
            ):
                for x_d, which in ((xq_d, "q"), (xk_d, "k")):
                    for sb in range(4):  # 512-row slab
                        xin = xin_pool.tile([128, 4, D], F32R, tag="xin")
                        nc.sync.dma_start(
                            out=xin,
                            in_=x_d[512 * sb : 512 * (sb + 1), :].rearrange(
                                "(sc p) d -> p sc d", p=128
                            ),
                        )
                        slab = slab_pool.tile([128, DC, 512], F32R, tag="slab")
                        for dc in range(DC):
                            tp = tpsum_pool.tile([128, 512], F32R, tag="tp")
                            for i in range(4):
                                nc.tensor.transpose(
                                    tp[:, 128 * i : 128 * (i + 1)],
                                    xin[:, i, 128 * dc : 128 * (dc + 1)],
                                    ident,
                                )
                            if dc % 2 == 0:
                                nc.vector.tensor_copy(slab[:, dc, :], tp)
                            else:
                                nc.scalar.copy(slab[:, dc, :], tp)

                        if which == "q":
                            # QT = wq.T @ xqT
                            for fc in range(2):
                                pq = ppsum_pool.tile([128, 512], F32, tag="pp")
                                for dc in range(DC):
                                    nc.tensor.matmul(
                                        pq,
                                        lhsT=wq[:, dc, 128 * fc : 128 * (fc + 1)],
                                        rhs=slab[:, dc, :],
                                        start=(dc == 0),
                                        stop=(dc == DC - 1),
                                    )
                                nc.vector.tensor_copy(
                                    qt[:, fc, 512 * sb : 512 * (sb + 1)], pq
                                )
                        else:
                            # KT = wk.T @ xkT
                            for fc in range(2):
                                pk = ppsum_pool.tile([128, 512], F32, tag="pp")
                                for dc in range(DC):
                                    nc.tensor.matmul(
                                        pk,
                                        lhsT=wk[:, dc, 128 * fc : 128 * (fc + 1)],
                                        rhs=slab[:, dc, :],
                                        start=(dc == 0),
                                        stop=(dc == DC - 1),
                                    )
                                nc.vector.tensor_copy(
                                    kt[:, fc, 512 * sb : 512 * (sb + 1)], pk
                                )
                            # V = xkT.T @ wv   -> [k-seq, f]
                            for i in range(4):
                                kc = 4 * sb + i
                                pv = vpsum_pool.tile([128, FPC], F32, tag="vp")
                                for dc in range(DC):
                                    nc.tensor.matmul(
                                        pv,
                                        lhsT=slab[:, dc, 128 * i : 128 * (i + 1)],
                                        rhs=wv[:, dc, :],
                                        start=(dc == 0),
                                        stop=(dc == DC - 1),
                                    )
                                # scatter heads into vt (col 64 of each head
                                # stays 1.0 from the memset)
                                nc.vector.tensor_copy(
                                    vt[:, kc, :].rearrange(
                                        "p (h f) -> p h f", h=HPC
                                    )[:, :, 0:64],
                                    pv.rearrange("p (h f) -> p h f", h=HPC),
                                )

            # ---- phase 3: attention
            with (
                tc.tile_pool(name="stps", bufs=3, space="PSUM") as st_pool,
                tc.tile_pool(name="pvps", bufs=2, space="PSUM") as pv_pool,
                tc.tile_pool(name="pt", bufs=34) as pt_pool,
                tc.tile_pool(name="ostage", bufs=2) as out_pool,
                tc.tile_pool(name="small", bufs=4) as small_pool,
            ):
                for b in range(NB):
                    pts = {}
                    for c in range(4 * b + 4):
                        qs = max(0, c - 4 * b) * 128  # skip fully-masked q cols
                        width = 512 - qs
                        for p in range(2):  # head pair = feature chunk
                            st = st_pool.tile([128, 2, 512], F32, tag="st")
                            for hh in range(2):
                                lo, hi = 64 * hh, 64 * (hh + 1)
                                nc.tensor.matmul(
                                    st[:, hh, :],
                                    lhsT=kt[lo:hi, p, 128 * c : 128 * (c + 1)],
                                    rhs=qt[lo:hi, p, 512 * b : 512 * (b + 1)],
                                    start=True,
                                    stop=True,
                                )
                            if c >= 4 * b:
                                j = c - 4 * b
                                for hh in range(2):
                                    sl = st[:, hh, 128 * j : 128 * (j + 1)]
                                    nc.vector.tensor_add(sl, sl, diag_mask)
                            pt = pt_pool.tile([128, 2, width], BF16, tag="pt")
                            nc.scalar.activation(
                                pt,
                                st[:, :, qs:],
                                mybir.ActivationFunctionType.Exp,
                                bias=pad_bias[:, c : c + 1],
                                scale=1.0,
                            )
                            pts[(c, p)] = (pt, qs)

                    for j in range(4):
                        qc = 4 * b + j
                        ostage = out_pool.tile([128, FPC], F32, tag="os")
                        for h in range(HPC):
                            p, hh = h // 2, h % 2
                            po = pv_pool.tile([128, 65], F32, tag="po")
                            for c in range(qc + 1):
                                ptile, qs = pts[(c, p)]
                                lo = 128 * j - qs
                                nc.tensor.matmul(
                                    po,
                                    lhsT=ptile[:, hh, lo : lo + 128],
                                    rhs=vt[:, c, 65 * h : 65 * (h + 1)],
                                    start=(c == 0),
                                    stop=(c == qc),
                                )
                            rec = small_pool.tile([128, 1], F32, tag="rec")
                            nc.vector.reciprocal(rec, po[:, 64:65])
                            nc.vector.tensor_scalar_mul(
                                ostage[:, 64 * h : 64 * (h + 1)], po[:, 0:64], rec
                            )
                        nc.sync.dma_start(
                            out=out_d[128 * qc : 128 * (qc + 1), :], in_=ostage
                        )

    nc.compile()
    return nc


_NC_CACHE = None


def get_program():
    global _NC_CACHE
    if _NC_CACHE is None:
        _NC_CACHE = build_program()
    return _NC_CACHE


def make_in_maps(query, key, Wq, Wk, Wv, padding_mask):
    query = np.asarray(query, dtype=np.float32)
    key = np.asarray(key, dtype=np.float32)
    Wq = np.asarray(Wq, dtype=np.float32)
    Wk = np.asarray(Wk, dtype=np.float32)
    Wv = np.asarray(Wv, dtype=np.float32)
    padding_mask = np.asarray(padding_mask)

    in_maps = []
    for core in range(8):
        n, g = core // 4, core % 4
        valid = float((~padding_mask[n]).sum())
        inv_scale = 1.0 / np.sqrt(valid)
        sl = slice(g * FPC, (g + 1) * FPC)
        pad_bias = np.where(padding_mask[n], NEG, 0.0).astype(np.float32)
        in_maps.append(
            {
                "xq": np.ascontiguousarray(query[n]),
                "xk": np.ascontiguousarray(key[n]),
                "wq_t": np.ascontiguousarray((Wq[sl] * inv_scale).T),
                "wk_t": np.ascontiguousarray(Wk[sl].T),
                "wv_t": np.ascontiguousarray(Wv[sl].T),
                "pad_bias": np.ascontiguousarray(
                    pad_bias.reshape(SC, 128).T
                ),
                "ident": np.eye(128, dtype=np.float32),
            }
        )
    return in_maps


def kernel(query, key, Wq, Wk, Wv, mask, padding_mask, n_heads):
    nc = get_program()
    in_maps = make_in_maps(query, key, Wq, Wk, Wv, padding_mask)
    res = run_bass_kernel_spmd(nc, in_maps, core_ids=list(range(8)))
    out = np.empty((2, L, D), dtype=np.float32)
    for core in range(8):
        n, g = core // 4, core % 4
        out[n, :, g * FPC : (g + 1) * FPC] = res.results[core]["out"]
    return out


# revision 11
# speedup vs baseline: 680.0117x; 680.0117x over previous
"""Multi-head attention Trainium2 kernel (8 NeuronCores, SPMD).

Problem: N=2, Lq=Lk=2048, D=1024, H=16 heads, causal + padding mask,
score scaling = sqrt(#valid keys per sentence).

Sharding: core c -> (n = c // 4, g = c % 4): batch n, head group g of 4
heads (256 feature columns). No cross-core communication; the host
assembles the per-core [2048, 256] outputs into [2, 2048, 1024].

Per-core pipeline (all on one NeuronCore):
  1. xq/xk arrive twice: host-cast bf16 copies are DMA-transposed (xbar,
     2-byte dtype) straight into d-major SBUF tiles xqT/xkT.
  2. Projections (bf16 matmuls, fp32 PSUM): QT/KT [256f, 2048s] bf16 and
     V [2048s, 256f] -> packed bf16 Vtilde [k-chunk][128, 4*65] with a
     ones column per head (softmax denominators for free).
  3. Scores transposed ST[k, q] = KT-slice.T @ QT (row-tiled head pairs,
     two concurrent 64-contraction matmuls). Causal handled structurally
     (skip fully-masked k-chunks and fully-masked q-subblocks) plus one
     [128,128] strictly-lower-triangular additive mask on diagonal
     chunks. The padding mask enters as the scalar-engine activation
     bias (per-partition = per-key). exp on ACT writes bf16 P'T tiles.
     The 1/sqrt(valid) scaling is folded into Wq on the host.
  4. PV: out[q, 0:64]+sum[q] = P'T-chunk.T @ Vtilde (bf16), accumulated
     over k-chunks in PSUM; normalize by the ones-column; DMA out.
"""

import sys

sys.path.insert(0, "/opt/trn_rl_repo")

import numpy as np
import ml_dtypes

import concourse.tile as tile
from concourse import bacc, mybir
from concourse.bass_utils import run_bass_kernel_spmd

F32 = mybir.dt.float32
BF16 = mybir.dt.bfloat16

L = 2048          # sequence length (q and k)
D = 1024          # model dim
FPC = 256         # features per core (4 heads x 64)
HPC = 4           # heads per core
SC = L // 128     # 16 seq chunks of 128
DC = D // 128     # 8 d chunks of 128
NB = L // 512     # 4 q-blocks of 512
NEG = -1.0e9


def build_program():
    nc = bacc.Bacc("TRN2", target_bir_lowering=False, debug=False, num_devices=8)

    xq_d = nc.dram_tensor("xq_bf", [L, D], BF16, kind="ExternalInput").ap()
    xk_d = nc.dram_tensor("xk_bf", [L, D], BF16, kind="ExternalInput").ap()
    wq_d = nc.dram_tensor("wq_t", [D, FPC], BF16, kind="ExternalInput").ap()
    wk_d = nc.dram_tensor("wk_t", [D, FPC], BF16, kind="ExternalInput").ap()
    wv_d = nc.dram_tensor("wv_t", [D, FPC], BF16, kind="ExternalInput").ap()
    pb_d = nc.dram_tensor("pad_bias", [128, SC], F32, kind="ExternalInput").ap()
    out_d = nc.dram_tensor("out", [L, FPC], F32, kind="ExternalOutput").ap()

    with tile.TileContext(nc) as tc:
        with (
            tc.tile_pool(name="consts", bufs=1) as consts,
            tc.tile_pool(name="wpool", bufs=1) as wpool,
            tc.tile_pool(name="xt", bufs=1) as xt_pool,
            tc.tile_pool(name="qkv", bufs=1) as qkv,
            tc.tile_pool(name="pt", bufs=34) as pt_pool,
            tc.tile_pool(name="ostage", bufs=2) as out_pool,
            tc.tile_pool(name="small", bufs=4) as small_pool,
            tc.tile_pool(name="proj", bufs=2, space="PSUM") as proj_pool,
            tc.tile_pool(name="stps", bufs=2, space="PSUM") as st_pool,
            tc.tile_pool(name="pvps", bufs=2, space="PSUM") as pv_pool,
        ):
            # diag_mask[i, j] = NEG where j < i else 0  (strictly-lower tri)
            diag_mask = consts.tile([128, 128], F32)
            nc.gpsimd.memset(diag_mask, 0.0)
            nc.gpsimd.affine_select(
                out=diag_mask,
                in_=diag_mask,
                compare_op=mybir.AluOpType.is_ge,
                fill=NEG,
                base=0,
                pattern=[[1, 128]],
                channel_multiplier=-1,
            )
            pad_bias = consts.tile([128, SC], F32)
            nc.sync.dma_start(out=pad_bias, in_=pb_d)

            # weights: [128 (d within chunk), dc, f]
            wq = wpool.tile([128, DC, FPC], BF16)
            wk = wpool.tile([128, DC, FPC], BF16)
            wv = wpool.tile([128, DC, FPC], BF16)
            for w_sb, w_dr in ((wq, wq_d), (wk, wk_d), (wv, wv_d)):
                nc.sync.dma_start(
                    out=w_sb, in_=w_dr.rearrange("(dc p) f -> p dc f", p=128)
                )

            # ACT warmup: trigger the exp table load at t~0 so the first
            # real exp doesn't pay the ~2.7us LoadActFuncSet latency.
            warm = small_pool.tile([128, 1], F32, tag="warm")
            warm2 = small_pool.tile([128, 1], F32, tag="warm")
            nc.vector.memset(warm, 0.0)
            nc.scalar.activation(warm2, warm, mybir.ActivationFunctionType.Exp)

            # x transposed, d-major: [128 (d in chunk), dc, seq]
            # Emitted in consumption order: projections for slab sb need all
            # d-chunks of BOTH xq and xk for that slab.
            xqt = xt_pool.tile([128, DC, L], BF16)
            xkt = xt_pool.tile([128, DC, L], BF16)
            for sb in range(4):
                for x_d, x_t in ((xq_d, xqt), (xk_d, xkt)):
                    for dc in range(DC):
                        nc.sync.dma_start(
                            out=x_t[:, dc, 512 * sb : 512 * (sb + 1)],
                            in_=x_d[
                                512 * sb : 512 * (sb + 1),
                                128 * dc : 128 * (dc + 1),
                            ],
                            transpose=True,
                        )

            # projection outputs
            qt = qkv.tile([128, 2, L], BF16)   # [f within chunk, fc, q]
            kt = qkv.tile([128, 2, L], BF16)   # [f within chunk, fc, k]
            vt = qkv.tile([128, SC, HPC * 65], BF16)  # [k in chunk, kc, h*65+f]
            nc.vector.memset(vt, 1.0)  # ones columns (col 64 of each head)

            # ---- helpers -------------------------------------------------
            def proj_slab(sb):
                for fc in range(2):
                    pq = proj_pool.tile([128, 512], F32, tag="proj")
                    for dc in range(DC):
                        nc.tensor.matmul(
                            pq,
                            lhsT=wq[:, dc, 128 * fc : 128 * (fc + 1)],
                            rhs=xqt[:, dc, 512 * sb : 512 * (sb + 1)],
                            start=(dc == 0),
                            stop=(dc == DC - 1),
                        )
                    nc.vector.tensor_copy(qt[:, fc, 512 * sb : 512 * (sb + 1)], pq)
                for fc in range(2):
                    pk = proj_pool.tile([128, 512], F32, tag="proj")
                    for dc in range(DC):
                        nc.tensor.matmul(
                            pk,
                            lhsT=wk[:, dc, 128 * fc : 128 * (fc + 1)],
                            rhs=xkt[:, dc, 512 * sb : 512 * (sb + 1)],
                            start=(dc == 0),
                            stop=(dc == DC - 1),
                        )
                    nc.vector.tensor_copy(kt[:, fc, 512 * sb : 512 * (sb + 1)], pk)
                # V = xkT.T @ wv   -> [k-seq, f]
                for i in range(4):
                    kc = 4 * sb + i
                    pv = proj_pool.tile([128, 512], F32, tag="proj")
                    for dc in range(DC):
                        nc.tensor.matmul(
                            pv[:, 0:FPC],
                            lhsT=xkt[:, dc, 128 * kc : 128 * (kc + 1)],
                            rhs=wv[:, dc, :],
                            start=(dc == 0),
                            stop=(dc == DC - 1),
                        )
                    # scatter heads into vt (col 64 of each head stays 1.0)
                    nc.vector.tensor_copy(
                        vt[:, kc, :].rearrange("p (h f) -> p h f", h=HPC)[
                            :, :, 0:64
                        ],
                        pv[:, 0:FPC].rearrange("p (h f) -> p h f", h=HPC),
                    )

            def st_exp(b, c, pts):
                qs = max(0, c - 4 * b) * 128  # skip fully-masked q cols
                width = 512 - qs
                for p in range(2):  # head pair = feature chunk
                    st = st_pool.tile([128, 2, 512], F32, tag="st")
                    for hh in range(2):
                        lo, hi = 64 * hh, 64 * (hh + 1)
                        nc.tensor.matmul(
                            st[:, hh, :],
                            lhsT=kt[lo:hi, p, 128 * c : 128 * (c + 1)],
                            rhs=qt[lo:hi, p, 512 * b : 512 * (b + 1)],
                            start=True,
                            stop=True,
                        )
                    if c >= 4 * b:
                        j = c - 4 * b
                        for hh in range(2):
                            sl = st[:, hh, 128 * j : 128 * (j + 1)]
                            nc.vector.tensor_add(sl, sl, diag_mask)
                    pt = pt_pool.tile([128, 2, width], BF16, tag="pt")
                    nc.scalar.activation(
                        pt,
                        st[:, :, qs:],
                        mybir.ActivationFunctionType.Exp,
                        bias=pad_bias[:, c : c + 1],
                        scale=1.0,
                    )
                    pts[(c, p)] = (pt, qs)

            def pv_qchunk(b, j, pts):
                qc = 4 * b + j
                ostage = out_pool.tile([128, FPC], F32, tag="os")
                for h in range(HPC):
                    p, hh = h // 2, h % 2
                    po = pv_pool.tile([128, 65], F32, tag="po")
                    for c in range(qc + 1):
                        ptile, qs = pts[(c, p)]
                        lo = 128 * j - qs
                        nc.tensor.matmul(
                            po,
                            lhsT=ptile[:, hh, lo : lo + 128],
                            rhs=vt[:, c, 65 * h : 65 * (h + 1)],
                            start=(c == 0),
                            stop=(c == qc),
                        )
                    rec = small_pool.tile([128, 1], F32, tag="rec")
                    nc.vector.reciprocal(rec, po[:, 64:65])
                    nc.vector.tensor_scalar_mul(
                        ostage[:, 64 * h : 64 * (h + 1)], po[:, 0:64], rec
                    )
                nc.sync.dma_start(
                    out=out_d[128 * qc : 128 * (qc + 1), :], in_=ostage
                )

            # ---- interleaved schedule: projections feed attention blocks;
            # within a block, PV(j) is emitted right after its last needed
            # exp so the PE never waits a whole block on ACT.
            for b in range(NB):
                proj_slab(b)
                pts = {}
                for c in range(4 * b + 1):
                    st_exp(b, c, pts)
                pv_qchunk(b, 0, pts)
                for j in range(1, 4):
                    st_exp(b, 4 * b + j, pts)
                    pv_qchunk(b, j, pts)

    nc.compile()
    return nc


_NC_CACHE = None


def get_program():
    global _NC_CACHE
    if _NC_CACHE is None:
        _NC_CACHE = build_program()
    return _NC_CACHE


def make_in_maps(query, key, Wq, Wk, Wv, padding_mask):
    query = np.asarray(query, dtype=np.float32)
    key = np.asarray(key, dtype=np.float32)
    Wq = np.asarray(Wq, dtype=np.float32)
    Wk = np.asarray(Wk, dtype=np.float32)
    Wv = np.asarray(Wv, dtype=np.float32)
    padding_mask = np.asarray(padding_mask)
    bf = ml_dtypes.bfloat16

    in_maps = []
    for core in range(8):
        n, g = core // 4, core % 4
        valid = float((~padding_mask[n]).sum())
        inv_scale = 1.0 / np.sqrt(valid)
        sl = slice(g * FPC, (g + 1) * FPC)
        pad_bias = np.where(padding_mask[n], NEG, 0.0).astype(np.float32)
        in_maps.append(
            {
                "xq_bf": np.ascontiguousarray(query[n]).astype(bf),
                "xk_bf": np.ascontiguousarray(key[n]).astype(bf),
                "wq_t": np.ascontiguousarray((Wq[sl] * inv_scale).T).astype(bf),
                "wk_t": np.ascontiguousarray(Wk[sl].T).astype(bf),
                "wv_t": np.ascontiguousarray(Wv[sl].T).astype(bf),
                "pad_bias": np.ascontiguousarray(pad_bias.reshape(SC, 128).T),
            }
        )
    return in_maps


def kernel(query, key, Wq, Wk, Wv, mask, padding_mask, n_heads):
    nc = get_program()
    in_maps = make_in_maps(query, key, Wq, Wk, Wv, padding_mask)
    res = run_bass_kernel_spmd(nc, in_maps, core_ids=list(range(8)))
    out = np.empty((2, L, D), dtype=np.float32)
    for core in range(8):
        n, g = core // 4, core % 4
        out[n, :, g * FPC : (g + 1) * FPC] = res.results[core]["out"]
    return out
